# revision 10
# baseline (speedup 1.0000x reference)
"""FFQLinear Trainium2 kernel (8 NeuronCores, column-parallel, fp8 hybrid).

Computes out = x2d @ W + bias with W = (q_int - zero_point) * scale, where
scale / zero_point broadcast over the OUTPUT-column axis of the [D, D] code
matrix (so W[:, j] = (q[:, j] - zp[j]) * scale[j]).

Math used on device (zp is zero in this problem; a host-side exact rank-1
correction handles the general case): since scale is per-output-column,
    out[:, j] = (x2d @ q)[:, j] * scale[j] + bias[j].

Precision/speed design: the PE runs fp8(e4m3) matmuls at ~2x the 16-bit
rate via perf_mode=DoubleRow (two k-subtiles contracted per instruction).
One pure-fp8 pass has rel err ~2.2e-2 (x-quant 1.6e-2 + q-quant 1.45e-2 in
quadrature) -- just over the 2e-2 budget -- so a split-K hybrid is used:
  - NKD8/8 of K (k-groups of 512) in e4m3 DoubleRow: q is centered
    (q' = q - 128, |q'| <= 128 fits e4m3 with ulp <= 8) and x is cast to
    e4m3. The removed mean is restored exactly in the epilogue via
    out[m, :] += 128 * rowsum(x)[m] (host-computed f32 row sums, added as
    a per-partition scalar before the scale multiply).
  - the remaining (8-NKD8)/8 of K in fp16 (q' ints exact in fp16, x fp16
    rounding ~2e-4): essentially error-free.
With NKD8=6 the simulated rel err is 1.87e-2 (deterministic for the fixed
harness inputs) and PE work drops to 0.25 + 0.75/2 = 62.5% of the fp16
baseline's.

Sharding: column-parallel per the hint. Each of the 8 cores gets
  - x pre-transposed and pre-tiled on the host (contraction dim on SBUF
    partitions, contiguous per-partition DMA lines), replicated
  - a [K, 512] column shard of q', and [512] shards of scale/bias
  - the [M] f32 vector v = 128*rowsum(x), pretiled to per-partition form
and produces a [M, 512] f32 output shard. Host concatenates the shards.
"""

import sys
import time
import types

import numpy as np
import ml_dtypes

import concourse.bass as bass
import concourse.bacc as bacc
import concourse.mybir as mybir
import concourse.tile as tile

# bass_utils' axon trace path does an unguarded
# `from antenv.axon_hooks import get_axon_ntff_profile_hook`; some images
# lack that module. Provide a stub (hook=None -> tracing degrades
# gracefully) so a BASS_TRACE=1 environment can't crash the kernel.
try:
    import antenv.axon_hooks  # noqa: F401
except Exception:
    try:
        import antenv

        _stub = types.ModuleType("antenv.axon_hooks")
        _stub._HOOK = None
        _stub.set_axon_ntff_profile_hook = lambda h: setattr(_stub, "_HOOK", h)
        _stub.get_axon_ntff_profile_hook = lambda: _stub._HOOK
        sys.modules["antenv.axon_hooks"] = _stub
        antenv.axon_hooks = _stub
    except Exception:
        pass

# boot() skips hook registration when the image's antenv lacks axon_hooks;
# with the stub in place, install the same ctypes hook it would have used
# so trace=True yields NTFF profiles / HW exec times.
try:
    import antenv.axon_hooks as _ah

    if _ah.get_axon_ntff_profile_hook() is None:
        from trn_agent_boot.trn_boot import _ntff_profile_via_ctypes

        _hook = _ntff_profile_via_ctypes("/opt/axon/libaxon_pjrt.so")
        if _hook is not None:
            _ah.set_axon_ntff_profile_hook(_hook)
except Exception:
    pass

from concourse.bass_utils import run_bass_kernel_spmd

B, S, D = 2, 2048, 4096
M = B * S            # 4096 output rows
K = D                # 4096 contraction
N = D                # 4096 output cols
NCORES = 8
NS = N // NCORES     # 512 output cols per core

P = 128
KO = K // P          # 32 k-subtiles
M_CHUNK = 512        # rows per chunk (4 psum tiles of 128)
MT = M_CHUNK // P    # 4
NMC = M // M_CHUNK   # 8 m-chunks
KPD = 4              # k-subtiles per x DMA group
NKD = KO // KPD      # 8 k-dma groups total

NKD8 = 6             # k-dma groups done in fp8 DoubleRow (rest fp16)
DT16 = "fp16"        # PE dtype for the high-precision k-groups

F32 = mybir.dt.float32
F8 = mybir.dt.float8e4
NP8 = ml_dtypes.float8_e4m3  # TRN FP8_EXP4-compatible (max 240, RNE)

_CACHE: dict = {}


def _dt16(name: str):
    return mybir.dt.float16 if name == "fp16" else mybir.dt.bfloat16


def _np16(name: str):
    return np.float16 if name == "fp16" else ml_dtypes.bfloat16


def _build(nkd8: int, dt16_name: str) -> bass.Bass:
    assert 1 <= nkd8 <= NKD
    nkd16 = NKD - nkd8
    DT = _dt16(dt16_name)
    DR = mybir.MatmulPerfMode.DoubleRow
    # Bacc (not plain Bass): its compile() runs generate_event_semaphores,
    # which splits multi-wait DMAs to satisfy the 1-wait HW encoding limit.
    nc = bacc.Bacc(
        "TRN2", target_bir_lowering=False, debug=False, num_devices=NCORES
    )
    # Host-pretiled layouts: every DMA below reads a fully-contiguous
    # [P, KPD, *] block.
    xt8 = nc.dram_tensor(
        "xt8", [NMC * nkd8, P, KPD, M_CHUNK], F8, kind="ExternalInput"
    )
    qs8 = nc.dram_tensor("qs8", [nkd8, P, KPD, NS], F8, kind="ExternalInput")
    if nkd16:
        xt16 = nc.dram_tensor(
            "xt16", [NMC * nkd16, P, KPD, M_CHUNK], DT, kind="ExternalInput"
        )
        qs16 = nc.dram_tensor(
            "qs16", [nkd16, P, KPD, NS], DT, kind="ExternalInput"
        )
    vrow_d = nc.dram_tensor("vrow", [P, NMC * MT], F32, kind="ExternalInput")
    scale_d = nc.dram_tensor("scale", [NS], F32, kind="ExternalInput")
    bias_d = nc.dram_tensor("bias", [NS], F32, kind="ExternalInput")
    out_d = nc.dram_tensor("out", [M, NS], F32, kind="ExternalOutput")

    with tile.TileContext(nc) as tc:
        with (
            tc.tile_pool(name="const", bufs=1) as cpool,
            tc.tile_pool(name="x8load", bufs=12) as x8pool,
            tc.tile_pool(name="x0load", bufs=2) as x0pool,
            tc.tile_pool(name="x16load", bufs=5) as x16pool,
            tc.tile_pool(name="opool", bufs=4) as opool,
            tc.tile_pool(name="psum", bufs=8, space="PSUM") as ppool,
        ):
            # Resident q shard. The kd=0 fp8 group is split per k-PAIR
            # (the DoubleRow unit) so the very first matmul waits on a
            # 128KB DMA, not the full group; remaining groups are one
            # DMA each, emitted interleaved with the first m-chunk's x
            # loads.
            q0 = [cpool.tile([P, 2, NS], F8, name=f"q0_{j}") for j in range(2)]
            q8 = [None] + [
                cpool.tile([P, KPD, NS], F8, name=f"q8_{kd}")
                for kd in range(1, nkd8)
            ]
            q16 = [
                cpool.tile([P, KPD, NS], DT, name=f"q16_{kd}")
                for kd in range(nkd16)
            ]
            scale_sb = cpool.tile([P, NS], F32)
            bias_sb = cpool.tile([P, NS], F32)
            v_sb = cpool.tile([P, NMC * MT], F32)

            def rhs8(kd, j):
                return q0[j][:] if kd == 0 else q8[kd][:, 2 * j:2 * j + 2, :]

            def load_chunk(mc, first=False):
                """Issue all x DMAs for chunk mc (and, on the first call,
                interleave the resident q-group DMAs)."""
                x8tiles = []
                x16tiles = []
                for kd in range(nkd8):
                    if first and kd == 0:
                        # per-k-SUBTILE 64KB DMAs for the fastest start
                        pairs = []
                        for j in range(2):
                            x_sb = x0pool.tile(
                                [P, 2, M_CHUNK], F8, name=f"x0_{j}", tag="x0"
                            )
                            for kk in range(2):
                                nc.sync.dma_start(
                                    q0[j][:, kk, :],
                                    qs8[0][:, 2 * j + kk, :],
                                )
                                nc.sync.dma_start(
                                    x_sb[:, kk, :], xt8[0][:, 2 * j + kk, :]
                                )
                            pairs.append(x_sb)
                        x8tiles.append(pairs)
                        continue
                    xts = x8pool.tile(
                        [P, KPD, M_CHUNK], F8, name="x8sb", tag="x8"
                    )
                    nc.sync.dma_start(xts[:], xt8[mc * nkd8 + kd])
                    x8tiles.append(xts)
                    if first:
                        nc.sync.dma_start(q8[kd][:], qs8[kd])
                for kd in range(nkd16):
                    xts = x16pool.tile(
                        [P, KPD, M_CHUNK], DT, name="x16sb", tag="x16"
                    )
                    nc.sync.dma_start(xts[:], xt16[mc * nkd16 + kd])
                    x16tiles.append(xts)
                    if first:
                        nc.sync.dma_start(q16[kd][:], qs16[kd])
                return x8tiles, x16tiles

            def lhs8_of(x8tiles, kd, j, mt):
                t = x8tiles[kd]
                if isinstance(t, list):  # (mc=0, kd=0) pair tiles
                    return t[j][:, :, mt * P:(mt + 1) * P]
                return t[:, 2 * j:2 * j + 2, mt * P:(mt + 1) * P]

            def mm8(psum, x8tiles, kd, j, mt):
                kp = kd * 2 + j
                nc.tensor.matmul(
                    psum[:],
                    lhsT=lhs8_of(x8tiles, kd, j, mt),
                    rhs=rhs8(kd, j),
                    start=(kp == 0),
                    stop=(nkd16 == 0 and kd == nkd8 - 1 and j == 1),
                    perf_mode=DR,
                )

            def mm16(psum, x16tiles, kd, kk, mt):
                nc.tensor.matmul(
                    psum[:],
                    lhsT=x16tiles[kd][:, kk, mt * P:(mt + 1) * P],
                    rhs=q16[kd][:, kk, :],
                    start=False,
                    stop=(kd == nkd16 - 1 and kk == KPD - 1),
                )

            def epilogue(psum, mc, mt, nh=1):
                # (psum + v) * scale + bias: the v-add runs on the Scalar
                # engine (per-partition bias), the rest on DVE, so the two
                # engines pipeline across tiles; the kernel-tail epilogues
                # run in column slices so DVE work overlaps the out DMAs.
                idx = mc * MT + mt
                row = idx * P
                H = NS // nh
                for h in range(nh):
                    o_sb = opool.tile(
                        [P, H], F32, name=f"osb{nh}{h}", tag=f"o{nh}{h}"
                    )
                    cs = slice(h * H, (h + 1) * H)
                    nc.scalar.activation(
                        o_sb[:], psum[:, cs],
                        mybir.ActivationFunctionType.Identity,
                        bias=v_sb[:, idx:idx + 1], scale=1.0,
                    )
                    nc.vector.tensor_mul(o_sb[:], o_sb[:], scale_sb[:, cs])
                    nc.vector.tensor_add(o_sb[:], o_sb[:], bias_sb[:, cs])
                    nc.sync.dma_start(out_d[row:row + P, cs], o_sb[:])

            cur = load_chunk(0, first=True)
            for mc in range(NMC):
                psums = [
                    ppool.tile([P, NS], F32, name=f"ps{mt}", tag="ps")
                    for mt in range(MT)
                ]
                last_mc = mc == NMC - 1
                if not last_mc:
                    nxt = load_chunk(mc + 1)
                if mc == 0:
                    nc.sync.dma_start(
                        scale_sb[:], scale_d[None, :].to_broadcast((P, NS))
                    )
                    nc.sync.dma_start(
                        bias_sb[:], bias_d[None, :].to_broadcast((P, NS))
                    )
                    nc.sync.dma_start(v_sb[:], vrow_d[:])
                x8tiles, x16tiles = cur
                if last_mc:
                    # mt-major: each psum finishes (and drains through the
                    # epilogue) while later mt groups still compute, so only
                    # one tile's epilogue trails the final matmul.
                    for mt in range(MT):
                        for kd in range(nkd8):
                            for j in range(2):
                                mm8(psums[mt], x8tiles, kd, j, mt)
                        for kd in range(nkd16):
                            for kk in range(KPD):
                                mm16(psums[mt], x16tiles, kd, kk, mt)
                        epilogue(psums[mt], mc, mt, nh=(4 if mt == MT - 1 else 2))
                else:
                    for kd in range(nkd8):
                        for j in range(2):
                            for mt in range(MT):
                                mm8(psums[mt], x8tiles, kd, j, mt)
                    for kd in range(nkd16):
                        for kk in range(KPD):
                            for mt in range(MT):
                                mm16(psums[mt], x16tiles, kd, kk, mt)
                    for mt in range(MT):
                        epilogue(psums[mt], mc, mt)
                    cur = nxt
    nc.compile()
    return nc


def _get_nc(nkd8: int, dt16_name: str) -> bass.Bass:
    key = (nkd8, dt16_name)
    if key not in _CACHE:
        _CACHE[key] = _build(nkd8, dt16_name)
    return _CACHE[key]


def _pretile_x(xpart: np.ndarray, nkd: int) -> np.ndarray:
    """[M, nkd*KPD*P] -> [NMC*nkd, P, KPD, M_CHUNK] with
    XD[mc*nkd+kd, p, kk, m] = xpart[mc*M_CHUNK + m, (kd*KPD+kk)*P + p]."""
    v = xpart.reshape(NMC, M_CHUNK, nkd, KPD, P)
    v = v.transpose(0, 2, 4, 3, 1)  # (mc, kd, p, kk, m)
    return np.ascontiguousarray(v).reshape(NMC * nkd, P, KPD, M_CHUNK)


def _pretile_q(qpart: np.ndarray, nkd: int) -> np.ndarray:
    """[nkd*KPD*P, NS] -> [nkd, P, KPD, NS] with
    QD[kd, p, kk, n] = qpart[(kd*KPD+kk)*P + p, n]."""
    v = qpart.reshape(nkd, KPD, P, NS)
    return np.ascontiguousarray(v.transpose(0, 2, 1, 3))


def _prep_in_maps(x, q_int, scale, bias, nkd8, dt16_name):
    np16 = _np16(dt16_name)
    nkd16 = NKD - nkd8
    k8 = nkd8 * KPD * P
    x2d = np.ascontiguousarray(x.reshape(M, K)).astype(np.float32, copy=False)
    xt8 = _pretile_x(x2d[:, :k8].astype(NP8), nkd8)
    if nkd16:
        xt16 = _pretile_x(np.ascontiguousarray(x2d[:, k8:]).astype(np16),
                          nkd16)

    # v = 128 * rowsum(x): restores the q-centering exactly (q' = q - 128
    # on device; both the fp8 and fp16 k-ranges are centered).
    v = (128.0 * x2d.astype(np.float64).sum(axis=1)).astype(np.float32)
    vrow = np.ascontiguousarray(v.reshape(NMC * MT, P).T)

    qc = q_int.astype(np.float32) - 128.0   # [-128, 127], exact in f32
    scale_f = scale.astype(np.float32, copy=False)
    bias_f = bias.astype(np.float32, copy=False)

    in_maps = []
    for c in range(NCORES):
        qs = qc[:, c * NS:(c + 1) * NS]
        m = {
            "xt8": xt8,
            "qs8": _pretile_q(np.ascontiguousarray(qs[:k8]).astype(NP8),
                              nkd8),
            "vrow": vrow,
            "scale": np.ascontiguousarray(scale_f[c * NS:(c + 1) * NS]),
            "bias": np.ascontiguousarray(bias_f[c * NS:(c + 1) * NS]),
        }
        if nkd16:
            m["xt16"] = xt16
            m["qs16"] = _pretile_q(
                np.ascontiguousarray(qs[k8:]).astype(np16), nkd16
            )
        in_maps.append(m)
    return in_maps


def _run(x, q_int, scale, zero_point, bias, nkd8=None, dt16_name=None,
         trace=False, **trace_kw):
    nkd8 = NKD8 if nkd8 is None else nkd8
    dt16_name = dt16_name or DT16
    nc = _get_nc(nkd8, dt16_name)
    in_maps = _prep_in_maps(x, q_int, scale, bias, nkd8, dt16_name)
    res = run_bass_kernel_spmd(
        nc, in_maps, list(range(NCORES)), trace=trace, **trace_kw
    )
    out2d = np.concatenate([r["out"] for r in res.results], axis=1)

    if np.any(np.asarray(zero_point) != 0):
        # exact rank-1 correction: -= rowsum(x) ⊗ (scale * zp)
        x2d = x.reshape(M, K).astype(np.float32, copy=False)
        out2d = out2d - np.outer(
            x2d.sum(axis=1),
            scale.astype(np.float32) * zero_point.astype(np.float32),
        )

    return out2d.reshape(B, S, D).astype(np.float32, copy=False), res


def _run_subprocess(x, q_int, scale, zero_point, bias):
    """Fresh-process retry: a NRT_EXEC_UNIT_UNRECOVERABLE poisons the
    in-process PJRT client, but a new process recovers."""
    import os
    import subprocess
    import tempfile

    d = tempfile.mkdtemp(prefix="ffq_retry_")
    names = ["x", "q_int", "scale", "zero_point", "bias"]
    for name, arr in zip(names, [x, q_int, scale, zero_point, bias]):
        np.save(os.path.join(d, name + ".npy"), np.asarray(arr))
    kdir = os.path.dirname(os.path.abspath(__file__))
    code = (
        "import sys, numpy as np\n"
        f"sys.path.insert(0, {kdir!r})\n"
        "import kernel as km\n"
        f"d = {d!r}\n"
        "ins = [np.load(d + '/' + n + '.npy') for n in "
        "['x', 'q_int', 'scale', 'zero_point', 'bias']]\n"
        "out, _ = km._run(*ins)\n"
        "np.save(d + '/out.npy', out)\n"
    )
    subprocess.run([sys.executable, "-c", code], check=True, timeout=2400)
    return np.load(os.path.join(d, "out.npy"))


def kernel(x, q_int, scale, zero_point, bias):
    try:
        out, _ = _run(x, q_int, scale, zero_point, bias)
    except Exception:
        # transient device errors (e.g. a core wedged by a previous
        # profiling session): retry in-process, then in a fresh process
        time.sleep(5)
        try:
            out, _ = _run(x, q_int, scale, zero_point, bias)
        except Exception:
            out = _run_subprocess(x, q_int, scale, zero_point, bias)
    return out


# revision 11
# speedup vs baseline: 1.0321x; 1.0321x over previous
"""FFQLinear Trainium2 kernel (8 NeuronCores, column-parallel, fp8 hybrid).

Computes out = x2d @ W + bias with W = (q_int - zero_point) * scale, where
scale / zero_point broadcast over the OUTPUT-column axis of the [D, D] code
matrix (so W[:, j] = (q[:, j] - zp[j]) * scale[j]).

Math used on device (zp is zero in this problem; a host-side exact rank-1
correction handles the general case): since scale is per-output-column,
    out[:, j] = (x2d @ q)[:, j] * scale[j] + bias[j].

Precision/speed design: the PE runs fp8(e4m3) matmuls at ~2x the 16-bit
rate via perf_mode=DoubleRow (two k-subtiles contracted per instruction).
One pure-fp8 pass has rel err ~2.2e-2 (x-quant 1.6e-2 + q-quant 1.45e-2 in
quadrature) -- just over the 2e-2 budget -- so a split-K hybrid is used:
  - NKD8/8 of K (k-groups of 512) in e4m3 DoubleRow: q is centered
    (q' = q - 128, |q'| <= 128 fits e4m3 with ulp <= 8) and x is cast to
    e4m3. The removed mean is restored exactly in the epilogue via
    out[m, :] += 128 * rowsum(x)[m] (host-computed f32 row sums, added as
    a per-partition scalar before the scale multiply).
  - the remaining (8-NKD8)/8 of K in fp16 (q' ints exact in fp16, x fp16
    rounding ~2e-4): essentially error-free.
With NKD8=6 the simulated rel err is 1.87e-2 (deterministic for the fixed
harness inputs) and PE work drops to 0.25 + 0.75/2 = 62.5% of the fp16
baseline's.

Sharding: column-parallel per the hint. Each of the 8 cores gets
  - x pre-transposed and pre-tiled on the host (contraction dim on SBUF
    partitions, contiguous per-partition DMA lines), replicated
  - a [K, 512] column shard of q', and [512] shards of scale/bias
  - the [M] f32 vector v = 128*rowsum(x), pretiled to per-partition form
and produces a [M, 512] f32 output shard. Host concatenates the shards.
"""

import sys
import time
import types

import numpy as np
import ml_dtypes

import concourse.bass as bass
import concourse.bacc as bacc
import concourse.mybir as mybir
import concourse.tile as tile

# bass_utils' axon trace path does an unguarded
# `from antenv.axon_hooks import get_axon_ntff_profile_hook`; some images
# lack that module. Provide a stub (hook=None -> tracing degrades
# gracefully) so a BASS_TRACE=1 environment can't crash the kernel.
try:
    import antenv.axon_hooks  # noqa: F401
except Exception:
    try:
        import antenv

        _stub = types.ModuleType("antenv.axon_hooks")
        _stub._HOOK = None
        _stub.set_axon_ntff_profile_hook = lambda h: setattr(_stub, "_HOOK", h)
        _stub.get_axon_ntff_profile_hook = lambda: _stub._HOOK
        sys.modules["antenv.axon_hooks"] = _stub
        antenv.axon_hooks = _stub
    except Exception:
        pass

# boot() skips hook registration when the image's antenv lacks axon_hooks;
# with the stub in place, install the same ctypes hook it would have used
# so trace=True yields NTFF profiles / HW exec times.
try:
    import antenv.axon_hooks as _ah

    if _ah.get_axon_ntff_profile_hook() is None:
        from trn_agent_boot.trn_boot import _ntff_profile_via_ctypes

        _hook = _ntff_profile_via_ctypes("/opt/axon/libaxon_pjrt.so")
        if _hook is not None:
            _ah.set_axon_ntff_profile_hook(_hook)
except Exception:
    pass

from concourse.bass_utils import run_bass_kernel_spmd

B, S, D = 2, 2048, 4096
M = B * S            # 4096 output rows
K = D                # 4096 contraction
N = D                # 4096 output cols
NCORES = 8
NS = N // NCORES     # 512 output cols per core

P = 128
KO = K // P          # 32 k-subtiles
M_CHUNK = 512        # rows per chunk (4 psum tiles of 128)
MT = M_CHUNK // P    # 4
NMC = M // M_CHUNK   # 8 m-chunks
KPD = 4              # k-subtiles per x DMA group
NKD = KO // KPD      # 8 k-dma groups total

NKD8 = 6             # k-dma groups done in fp8 DoubleRow (rest fp16)
DT16 = "fp16"        # PE dtype for the high-precision k-groups

F32 = mybir.dt.float32
F8 = mybir.dt.float8e4
NP8 = ml_dtypes.float8_e4m3  # TRN FP8_EXP4-compatible (max 240, RNE)

_CACHE: dict = {}


def _dt16(name: str):
    return mybir.dt.float16 if name == "fp16" else mybir.dt.bfloat16


def _np16(name: str):
    return np.float16 if name == "fp16" else ml_dtypes.bfloat16


def _build(nkd8: int, dt16_name: str) -> bass.Bass:
    assert 1 <= nkd8 <= NKD
    nkd16 = NKD - nkd8
    DT = _dt16(dt16_name)
    DR = mybir.MatmulPerfMode.DoubleRow
    # Bacc (not plain Bass): its compile() runs generate_event_semaphores,
    # which splits multi-wait DMAs to satisfy the 1-wait HW encoding limit.
    nc = bacc.Bacc(
        "TRN2", target_bir_lowering=False, debug=False, num_devices=NCORES
    )
    # Host-pretiled layouts: every DMA below reads a fully-contiguous
    # [P, KPD, *] block.
    xt8 = nc.dram_tensor(
        "xt8", [NMC * nkd8, P, KPD, M_CHUNK], F8, kind="ExternalInput"
    )
    qs8 = nc.dram_tensor("qs8", [nkd8, P, KPD, NS], F8, kind="ExternalInput")
    if nkd16:
        xt16 = nc.dram_tensor(
            "xt16", [NMC * nkd16, P, KPD, M_CHUNK], DT, kind="ExternalInput"
        )
        qs16 = nc.dram_tensor(
            "qs16", [nkd16, P, KPD, NS], DT, kind="ExternalInput"
        )
    vrow_d = nc.dram_tensor("vrow", [P, NMC * MT], F32, kind="ExternalInput")
    scale_d = nc.dram_tensor("scale", [NS], F32, kind="ExternalInput")
    bias_d = nc.dram_tensor("bias", [NS], F32, kind="ExternalInput")
    out_d = nc.dram_tensor("out", [M, NS], F32, kind="ExternalOutput")

    with tile.TileContext(nc) as tc:
        with (
            tc.tile_pool(name="const", bufs=1) as cpool,
            tc.tile_pool(name="x8load", bufs=12) as x8pool,
            tc.tile_pool(name="x0load", bufs=2) as x0pool,
            tc.tile_pool(name="x16load", bufs=5) as x16pool,
            tc.tile_pool(name="opool", bufs=4) as opool,
            tc.tile_pool(name="psum", bufs=8, space="PSUM") as ppool,
        ):
            # Resident q shard. The kd=0 fp8 group is split per k-PAIR
            # (the DoubleRow unit) so the very first matmul waits on a
            # 128KB DMA, not the full group; remaining groups are one
            # DMA each, emitted interleaved with the first m-chunk's x
            # loads.
            q0 = [cpool.tile([P, 2, NS], F8, name=f"q0_{j}") for j in range(2)]
            q8 = [None] + [
                cpool.tile([P, KPD, NS], F8, name=f"q8_{kd}")
                for kd in range(1, nkd8)
            ]
            q16 = [
                cpool.tile([P, KPD, NS], DT, name=f"q16_{kd}")
                for kd in range(nkd16)
            ]
            scale_sb = cpool.tile([P, NS], F32)
            bias_sb = cpool.tile([P, NS], F32)
            v_sb = cpool.tile([P, NMC * MT], F32)

            def rhs8(kd, j):
                return q0[j][:] if kd == 0 else q8[kd][:, 2 * j:2 * j + 2, :]

            def load_chunk(mc, first=False):
                """Issue all x DMAs for chunk mc (and, on the first call,
                interleave the resident q-group DMAs)."""
                x8tiles = []
                x16tiles = []
                for kd in range(nkd8):
                    if first and kd == 0:
                        # per-k-pair 128KB DMAs for the fastest start
                        pairs = []
                        for j in range(2):
                            nc.sync.dma_start(
                                q0[j][:], qs8[0][:, 2 * j:2 * j + 2, :]
                            )
                            x_sb = x0pool.tile(
                                [P, 2, M_CHUNK], F8, name=f"x0_{j}", tag="x0"
                            )
                            nc.sync.dma_start(
                                x_sb[:], xt8[0][:, 2 * j:2 * j + 2, :]
                            )
                            pairs.append(x_sb)
                        x8tiles.append(pairs)
                        continue
                    xts = x8pool.tile(
                        [P, KPD, M_CHUNK], F8, name="x8sb", tag="x8"
                    )
                    nc.sync.dma_start(xts[:], xt8[mc * nkd8 + kd])
                    x8tiles.append(xts)
                    if first:
                        nc.sync.dma_start(q8[kd][:], qs8[kd])
                for kd in range(nkd16):
                    xts = x16pool.tile(
                        [P, KPD, M_CHUNK], DT, name="x16sb", tag="x16"
                    )
                    nc.sync.dma_start(xts[:], xt16[mc * nkd16 + kd])
                    x16tiles.append(xts)
                    if first:
                        nc.sync.dma_start(q16[kd][:], qs16[kd])
                return x8tiles, x16tiles

            def lhs8_of(x8tiles, kd, j, mt):
                t = x8tiles[kd]
                if isinstance(t, list):  # (mc=0, kd=0) pair tiles
                    return t[j][:, :, mt * P:(mt + 1) * P]
                return t[:, 2 * j:2 * j + 2, mt * P:(mt + 1) * P]

            def mm8(psum, x8tiles, kd, j, mt):
                kp = kd * 2 + j
                nc.tensor.matmul(
                    psum[:],
                    lhsT=lhs8_of(x8tiles, kd, j, mt),
                    rhs=rhs8(kd, j),
                    start=(kp == 0),
                    stop=(nkd16 == 0 and kd == nkd8 - 1 and j == 1),
                    perf_mode=DR,
                )

            def mm16(psum, x16tiles, kd, kk, mt):
                nc.tensor.matmul(
                    psum[:],
                    lhsT=x16tiles[kd][:, kk, mt * P:(mt + 1) * P],
                    rhs=q16[kd][:, kk, :],
                    start=False,
                    stop=(kd == nkd16 - 1 and kk == KPD - 1),
                )

            def epilogue(psum, mc, mt, nh=1):
                # (psum + v) * scale + bias: the v-add runs on the Scalar
                # engine (per-partition bias), the rest on DVE, so the two
                # engines pipeline across tiles; the kernel-tail epilogues
                # run in column slices so DVE work overlaps the out DMAs.
                idx = mc * MT + mt
                row = idx * P
                H = NS // nh
                for h in range(nh):
                    o_sb = opool.tile(
                        [P, H], F32, name=f"osb{nh}{h}", tag=f"o{nh}{h}"
                    )
                    cs = slice(h * H, (h + 1) * H)
                    nc.scalar.activation(
                        o_sb[:], psum[:, cs],
                        mybir.ActivationFunctionType.Identity,
                        bias=v_sb[:, idx:idx + 1], scale=1.0,
                    )
                    nc.vector.tensor_mul(o_sb[:], o_sb[:], scale_sb[:, cs])
                    nc.vector.tensor_add(o_sb[:], o_sb[:], bias_sb[:, cs])
                    nc.sync.dma_start(out_d[row:row + P, cs], o_sb[:])

            cur = load_chunk(0, first=True)
            for mc in range(NMC):
                psums = [
                    ppool.tile([P, NS], F32, name=f"ps{mt}", tag="ps")
                    for mt in range(MT)
                ]
                last_mc = mc == NMC - 1
                if not last_mc:
                    nxt = load_chunk(mc + 1)
                if mc == 0:
                    nc.sync.dma_start(
                        scale_sb[:], scale_d[None, :].to_broadcast((P, NS))
                    )
                    nc.sync.dma_start(
                        bias_sb[:], bias_d[None, :].to_broadcast((P, NS))
                    )
                    nc.sync.dma_start(v_sb[:], vrow_d[:])
                x8tiles, x16tiles = cur
                if last_mc:
                    # mt-major: each psum finishes (and drains through the
                    # epilogue) while later mt groups still compute, so only
                    # one tile's epilogue trails the final matmul.
                    for mt in range(MT):
                        for kd in range(nkd8):
                            for j in range(2):
                                mm8(psums[mt], x8tiles, kd, j, mt)
                        for kd in range(nkd16):
                            for kk in range(KPD):
                                mm16(psums[mt], x16tiles, kd, kk, mt)
                        epilogue(psums[mt], mc, mt, nh=(4 if mt == MT - 1 else 2))
                else:
                    for kd in range(nkd8):
                        for j in range(2):
                            for mt in range(MT):
                                mm8(psums[mt], x8tiles, kd, j, mt)
                    for kd in range(nkd16):
                        for kk in range(KPD):
                            for mt in range(MT):
                                mm16(psums[mt], x16tiles, kd, kk, mt)
                    for mt in range(MT):
                        epilogue(psums[mt], mc, mt)
                    cur = nxt
    nc.compile()
    return nc


def _get_nc(nkd8: int, dt16_name: str) -> bass.Bass:
    key = (nkd8, dt16_name)
    if key not in _CACHE:
        _CACHE[key] = _build(nkd8, dt16_name)
    return _CACHE[key]


def _pretile_x(xpart: np.ndarray, nkd: int) -> np.ndarray:
    """[M, nkd*KPD*P] -> [NMC*nkd, P, KPD, M_CHUNK] with
    XD[mc*nkd+kd, p, kk, m] = xpart[mc*M_CHUNK + m, (kd*KPD+kk)*P + p]."""
    v = xpart.reshape(NMC, M_CHUNK, nkd, KPD, P)
    v = v.transpose(0, 2, 4, 3, 1)  # (mc, kd, p, kk, m)
    return np.ascontiguousarray(v).reshape(NMC * nkd, P, KPD, M_CHUNK)


def _pretile_q(qpart: np.ndarray, nkd: int) -> np.ndarray:
    """[nkd*KPD*P, NS] -> [nkd, P, KPD, NS] with
    QD[kd, p, kk, n] = qpart[(kd*KPD+kk)*P + p, n]."""
    v = qpart.reshape(nkd, KPD, P, NS)
    return np.ascontiguousarray(v.transpose(0, 2, 1, 3))


def _prep_in_maps(x, q_int, scale, bias, nkd8, dt16_name):
    np16 = _np16(dt16_name)
    nkd16 = NKD - nkd8
    k8 = nkd8 * KPD * P
    x2d = np.ascontiguousarray(x.reshape(M, K)).astype(np.float32, copy=False)
    xt8 = _pretile_x(x2d[:, :k8].astype(NP8), nkd8)
    if nkd16:
        xt16 = _pretile_x(np.ascontiguousarray(x2d[:, k8:]).astype(np16),
                          nkd16)

    # v = 128 * rowsum(x): restores the q-centering exactly (q' = q - 128
    # on device; both the fp8 and fp16 k-ranges are centered).
    v = (128.0 * x2d.astype(np.float64).sum(axis=1)).astype(np.float32)
    vrow = np.ascontiguousarray(v.reshape(NMC * MT, P).T)

    qc = q_int.astype(np.float32) - 128.0   # [-128, 127], exact in f32
    scale_f = scale.astype(np.float32, copy=False)
    bias_f = bias.astype(np.float32, copy=False)

    in_maps = []
    for c in range(NCORES):
        qs = qc[:, c * NS:(c + 1) * NS]
        m = {
            "xt8": xt8,
            "qs8": _pretile_q(np.ascontiguousarray(qs[:k8]).astype(NP8),
                              nkd8),
            "vrow": vrow,
            "scale": np.ascontiguousarray(scale_f[c * NS:(c + 1) * NS]),
            "bias": np.ascontiguousarray(bias_f[c * NS:(c + 1) * NS]),
        }
        if nkd16:
            m["xt16"] = xt16
            m["qs16"] = _pretile_q(
                np.ascontiguousarray(qs[k8:]).astype(np16), nkd16
            )
        in_maps.append(m)
    return in_maps


def _run(x, q_int, scale, zero_point, bias, nkd8=None, dt16_name=None,
         trace=False, **trace_kw):
    nkd8 = NKD8 if nkd8 is None else nkd8
    dt16_name = dt16_name or DT16
    nc = _get_nc(nkd8, dt16_name)
    in_maps = _prep_in_maps(x, q_int, scale, bias, nkd8, dt16_name)
    res = run_bass_kernel_spmd(
        nc, in_maps, list(range(NCORES)), trace=trace, **trace_kw
    )
    out2d = np.concatenate([r["out"] for r in res.results], axis=1)

    if np.any(np.asarray(zero_point) != 0):
        # exact rank-1 correction: -= rowsum(x) ⊗ (scale * zp)
        x2d = x.reshape(M, K).astype(np.float32, copy=False)
        out2d = out2d - np.outer(
            x2d.sum(axis=1),
            scale.astype(np.float32) * zero_point.astype(np.float32),
        )

    return out2d.reshape(B, S, D).astype(np.float32, copy=False), res


def _run_subprocess(x, q_int, scale, zero_point, bias):
    """Fresh-process retry: a NRT_EXEC_UNIT_UNRECOVERABLE poisons the
    in-process PJRT client, but a new process recovers."""
    import os
    import subprocess
    import tempfile

    d = tempfile.mkdtemp(prefix="ffq_retry_")
    names = ["x", "q_int", "scale", "zero_point", "bias"]
    for name, arr in zip(names, [x, q_int, scale, zero_point, bias]):
        np.save(os.path.join(d, name + ".npy"), np.asarray(arr))
    kdir = os.path.dirname(os.path.abspath(__file__))
    code = (
        "import sys, numpy as np\n"
        f"sys.path.insert(0, {kdir!r})\n"
        "import kernel as km\n"
        f"d = {d!r}\n"
        "ins = [np.load(d + '/' + n + '.npy') for n in "
        "['x', 'q_int', 'scale', 'zero_point', 'bias']]\n"
        "out, _ = km._run(*ins)\n"
        "np.save(d + '/out.npy', out)\n"
    )
    subprocess.run([sys.executable, "-c", code], check=True, timeout=2400)
    return np.load(os.path.join(d, "out.npy"))


def kernel(x, q_int, scale, zero_point, bias):
    try:
        out, _ = _run(x, q_int, scale, zero_point, bias)
    except Exception:
        # transient device errors (e.g. a core wedged by a previous
        # profiling session): retry in-process, then in a fresh process
        time.sleep(5)
        try:
            out, _ = _run(x, q_int, scale, zero_point, bias)
        except Exception:
            out = _run_subprocess(x, q_int, scale, zero_point, bias)
    return out


# revision 14
# speedup vs baseline: 1.0414x; 1.0090x over previous
"""FFQLinear Trainium2 kernel (8 NeuronCores, column-parallel, fp8 hybrid).

Computes out = x2d @ W + bias with W = (q_int - zero_point) * scale, where
scale / zero_point broadcast over the OUTPUT-column axis of the [D, D] code
matrix (so W[:, j] = (q[:, j] - zp[j]) * scale[j]).

Math used on device (zp is zero in this problem; a host-side exact rank-1
correction handles the general case): since scale is per-output-column,
    out[:, j] = (x2d @ q)[:, j] * scale[j] + bias[j].

Precision/speed design: the PE runs fp8(e4m3) matmuls at ~2x the 16-bit
rate via perf_mode=DoubleRow (two k-subtiles contracted per instruction).
One pure-fp8 pass has rel err ~2.2e-2 (x-quant 1.6e-2 + q-quant 1.45e-2 in
quadrature) -- just over the 2e-2 budget -- so a split-K hybrid is used:
  - NKD8/8 of K (k-groups of 512) in e4m3 DoubleRow: q is centered
    (q' = q - 128, |q'| <= 128 fits e4m3 with ulp <= 8) and x is cast to
    e4m3. The removed mean is restored exactly in the epilogue via
    out[m, :] += 128 * rowsum(x)[m] (host-computed f32 row sums, added as
    a per-partition scalar before the scale multiply).
  - the remaining (8-NKD8)/8 of K in fp16 (q' ints exact in fp16, x fp16
    rounding ~2e-4): essentially error-free.
With NKD8=6 the simulated rel err is 1.87e-2 (deterministic for the fixed
harness inputs) and PE work drops to 0.25 + 0.75/2 = 62.5% of the fp16
baseline's.

Sharding: column-parallel per the hint. Each of the 8 cores gets
  - x pre-transposed and pre-tiled on the host (contraction dim on SBUF
    partitions, contiguous per-partition DMA lines), replicated
  - a [K, 512] column shard of q', and [512] shards of scale/bias
  - the [M] f32 vector v = 128*rowsum(x), pretiled to per-partition form
and produces a [M, 512] f32 output shard. Host concatenates the shards.
"""

import sys
import time
import types

import numpy as np
import ml_dtypes

import concourse.bass as bass
import concourse.bacc as bacc
import concourse.mybir as mybir
import concourse.tile as tile

# bass_utils' axon trace path does an unguarded
# `from antenv.axon_hooks import get_axon_ntff_profile_hook`; some images
# lack that module. Provide a stub (hook=None -> tracing degrades
# gracefully) so a BASS_TRACE=1 environment can't crash the kernel.
try:
    import antenv.axon_hooks  # noqa: F401
except Exception:
    try:
        import antenv

        _stub = types.ModuleType("antenv.axon_hooks")
        _stub._HOOK = None
        _stub.set_axon_ntff_profile_hook = lambda h: setattr(_stub, "_HOOK", h)
        _stub.get_axon_ntff_profile_hook = lambda: _stub._HOOK
        sys.modules["antenv.axon_hooks"] = _stub
        antenv.axon_hooks = _stub
    except Exception:
        pass

# boot() skips hook registration when the image's antenv lacks axon_hooks;
# with the stub in place, install the same ctypes hook it would have used
# so trace=True yields NTFF profiles / HW exec times.
try:
    import antenv.axon_hooks as _ah

    if _ah.get_axon_ntff_profile_hook() is None:
        from trn_agent_boot.trn_boot import _ntff_profile_via_ctypes

        _hook = _ntff_profile_via_ctypes("/opt/axon/libaxon_pjrt.so")
        if _hook is not None:
            _ah.set_axon_ntff_profile_hook(_hook)
except Exception:
    pass

from concourse.bass_utils import run_bass_kernel_spmd

B, S, D = 2, 2048, 4096
M = B * S            # 4096 output rows
K = D                # 4096 contraction
N = D                # 4096 output cols
NCORES = 8
NS = N // NCORES     # 512 output cols per core

P = 128
KO = K // P          # 32 k-subtiles
M_CHUNK = 512        # rows per chunk (4 psum tiles of 128)
MT = M_CHUNK // P    # 4
NMC = M // M_CHUNK   # 8 m-chunks
KPD = 4              # k-subtiles per x DMA group
NKD = KO // KPD      # 8 k-dma groups total

NKD8 = 6             # k-dma groups done in fp8 DoubleRow (rest fp16)
DT16 = "fp16"        # PE dtype for the high-precision k-groups

F32 = mybir.dt.float32
F8 = mybir.dt.float8e4
NP8 = ml_dtypes.float8_e4m3  # TRN FP8_EXP4-compatible (max 240, RNE)

_CACHE: dict = {}


def _dt16(name: str):
    return mybir.dt.float16 if name == "fp16" else mybir.dt.bfloat16


def _np16(name: str):
    return np.float16 if name == "fp16" else ml_dtypes.bfloat16


def _build(nkd8: int, dt16_name: str) -> bass.Bass:
    assert 1 <= nkd8 <= NKD
    nkd16 = NKD - nkd8
    DT = _dt16(dt16_name)
    DR = mybir.MatmulPerfMode.DoubleRow
    # Bacc (not plain Bass): its compile() runs generate_event_semaphores,
    # which splits multi-wait DMAs to satisfy the 1-wait HW encoding limit.
    nc = bacc.Bacc(
        "TRN2", target_bir_lowering=False, debug=False, num_devices=NCORES
    )
    # Host-pretiled layouts: every DMA below reads a fully-contiguous
    # [P, KPD, *] block.
    xt8 = nc.dram_tensor(
        "xt8", [NMC * nkd8, P, KPD, M_CHUNK], F8, kind="ExternalInput"
    )
    qs8 = nc.dram_tensor("qs8", [nkd8, P, KPD, NS], F8, kind="ExternalInput")
    if nkd16:
        xt16 = nc.dram_tensor(
            "xt16", [NMC * nkd16, P, KPD, M_CHUNK], DT, kind="ExternalInput"
        )
        qs16 = nc.dram_tensor(
            "qs16", [nkd16, P, KPD, NS], DT, kind="ExternalInput"
        )
    vrow_d = nc.dram_tensor("vrow", [P, NMC * MT], F32, kind="ExternalInput")
    scale_d = nc.dram_tensor("scale", [NS], F32, kind="ExternalInput")
    bias_d = nc.dram_tensor("bias", [NS], F32, kind="ExternalInput")
    out_d = nc.dram_tensor("out", [M, NS], F32, kind="ExternalOutput")

    with tile.TileContext(nc) as tc:
        with (
            tc.tile_pool(name="const", bufs=1) as cpool,
            tc.tile_pool(name="x8load", bufs=8) as x8pool,
            tc.tile_pool(name="x0load", bufs=2) as x0pool,
            tc.tile_pool(name="x16load", bufs=4) as x16pool,
            tc.tile_pool(name="opool", bufs=4) as opool,
            tc.tile_pool(name="psum", bufs=8, space="PSUM") as ppool,
        ):
            # Resident q shard. The kd=0 fp8 group is split per k-PAIR
            # (the DoubleRow unit) so the very first matmul waits on a
            # 128KB DMA, not the full group; remaining groups are one
            # DMA each, emitted interleaved with the first m-chunk's x
            # loads.
            q0 = [cpool.tile([P, 2, NS], F8, name=f"q0_{j}") for j in range(2)]
            q8 = [None] + [
                cpool.tile([P, KPD, NS], F8, name=f"q8_{kd}")
                for kd in range(1, nkd8)
            ]
            q16 = [
                cpool.tile([P, KPD, NS], DT, name=f"q16_{kd}")
                for kd in range(nkd16)
            ]
            scale_sb = cpool.tile([P, NS], F32)
            bias_sb = cpool.tile([P, NS], F32)
            v_sb = cpool.tile([P, NMC * MT], F32)

            def rhs8(kd, j):
                return q0[j][:] if kd == 0 else q8[kd][:, 2 * j:2 * j + 2, :]

            def lhs8_of(x8tiles, kd, j, mt):
                t = x8tiles[kd]
                if isinstance(t, list):  # (mc=0, kd=0) pair tiles
                    return t[j][:, :, mt * P:(mt + 1) * P]
                return t[:, 2 * j:2 * j + 2, mt * P:(mt + 1) * P]

            def mm8(psum, x8tiles, kd, j, mt):
                kp = kd * 2 + j
                nc.tensor.matmul(
                    psum[:],
                    lhsT=lhs8_of(x8tiles, kd, j, mt),
                    rhs=rhs8(kd, j),
                    start=(kp == 0),
                    stop=(nkd16 == 0 and kd == nkd8 - 1 and j == 1),
                    perf_mode=DR,
                )

            def mm16(psum, x16tiles, kd, kk, mt):
                nc.tensor.matmul(
                    psum[:],
                    lhsT=x16tiles[kd][:, kk, mt * P:(mt + 1) * P],
                    rhs=q16[kd][:, kk, :],
                    start=False,
                    stop=(kd == nkd16 - 1 and kk == KPD - 1),
                )

            def epilogue(psum, mc, mt, halves=False):
                idx = mc * MT + mt
                row = idx * P
                nh = 2 if halves else 1
                H = NS // nh
                for h in range(nh):
                    # pipeline the very last epilogue in column halves:
                    # DVE on half 1 overlaps the DMA of half 0,
                    # shortening the kernel tail
                    o_sb = opool.tile(
                        [P, H], F32, name=f"osb{nh}{h}", tag=f"o{nh}{h}"
                    )
                    cs = slice(h * H, (h + 1) * H)
                    nc.vector.tensor_scalar_add(
                        o_sb[:], psum[:, cs], v_sb[:, idx:idx + 1]
                    )
                    nc.vector.tensor_mul(o_sb[:], o_sb[:], scale_sb[:, cs])
                    nc.vector.tensor_add(o_sb[:], o_sb[:], bias_sb[:, cs])
                    nc.sync.dma_start(out_d[row:row + P, cs], o_sb[:])

            for mc in range(NMC):
                psums = [
                    ppool.tile([P, NS], F32, name=f"ps{mt}", tag="ps")
                    for mt in range(MT)
                ]
                last_mc = mc == NMC - 1
                first = mc == 0
                x8tiles = []
                x16tiles = []
                for kd in range(nkd8):
                    if first and kd == 0:
                        # per-k-pair 128KB DMAs for the fastest start
                        pairs = []
                        for j in range(2):
                            nc.sync.dma_start(
                                q0[j][:], qs8[0][:, 2 * j:2 * j + 2, :]
                            )
                            x_sb = x0pool.tile(
                                [P, 2, M_CHUNK], F8, name=f"x0_{j}", tag="x0"
                            )
                            nc.sync.dma_start(
                                x_sb[:], xt8[0][:, 2 * j:2 * j + 2, :]
                            )
                            pairs.append(x_sb)
                        x8tiles.append(pairs)
                    else:
                        if first:
                            nc.sync.dma_start(q8[kd][:], qs8[kd])
                        xts = x8pool.tile(
                            [P, KPD, M_CHUNK], F8, name="x8sb", tag="x8"
                        )
                        nc.sync.dma_start(xts[:], xt8[mc * nkd8 + kd])
                        x8tiles.append(xts)
                    if last_mc:
                        continue
                    for j in range(2):
                        for mt in range(MT):
                            mm8(psums[mt], x8tiles, kd, j, mt)
                for kd in range(nkd16):
                    if first:
                        nc.sync.dma_start(q16[kd][:], qs16[kd])
                    xts = x16pool.tile(
                        [P, KPD, M_CHUNK], DT, name="x16sb", tag="x16"
                    )
                    nc.sync.dma_start(xts[:], xt16[mc * nkd16 + kd])
                    x16tiles.append(xts)
                    if last_mc:
                        continue
                    for kk in range(KPD):
                        for mt in range(MT):
                            mm16(psums[mt], x16tiles, kd, kk, mt)
                if last_mc:
                    # mt-major: each psum finishes (and drains through the
                    # epilogue) while later mt groups still compute, so only
                    # one tile's epilogue trails the final matmul.
                    for mt in range(MT):
                        for kd in range(nkd8):
                            for j in range(2):
                                mm8(psums[mt], x8tiles, kd, j, mt)
                        for kd in range(nkd16):
                            for kk in range(KPD):
                                mm16(psums[mt], x16tiles, kd, kk, mt)
                        epilogue(psums[mt], mc, mt, halves=(mt == MT - 1))
                    continue
                if first:
                    nc.sync.dma_start(
                        scale_sb[:], scale_d[None, :].to_broadcast((P, NS))
                    )
                    nc.sync.dma_start(
                        bias_sb[:], bias_d[None, :].to_broadcast((P, NS))
                    )
                    nc.sync.dma_start(v_sb[:], vrow_d[:])
                for mt in range(MT):
                    epilogue(psums[mt], mc, mt)
    nc.compile()
    return nc


def _get_nc(nkd8: int, dt16_name: str) -> bass.Bass:
    key = (nkd8, dt16_name)
    if key not in _CACHE:
        _CACHE[key] = _build(nkd8, dt16_name)
    return _CACHE[key]


def _pretile_x(xpart: np.ndarray, nkd: int) -> np.ndarray:
    """[M, nkd*KPD*P] -> [NMC*nkd, P, KPD, M_CHUNK] with
    XD[mc*nkd+kd, p, kk, m] = xpart[mc*M_CHUNK + m, (kd*KPD+kk)*P + p]."""
    v = xpart.reshape(NMC, M_CHUNK, nkd, KPD, P)
    v = v.transpose(0, 2, 4, 3, 1)  # (mc, kd, p, kk, m)
    return np.ascontiguousarray(v).reshape(NMC * nkd, P, KPD, M_CHUNK)


def _pretile_q(qpart: np.ndarray, nkd: int) -> np.ndarray:
    """[nkd*KPD*P, NS] -> [nkd, P, KPD, NS] with
    QD[kd, p, kk, n] = qpart[(kd*KPD+kk)*P + p, n]."""
    v = qpart.reshape(nkd, KPD, P, NS)
    return np.ascontiguousarray(v.transpose(0, 2, 1, 3))


def _prep_in_maps(x, q_int, scale, bias, nkd8, dt16_name):
    np16 = _np16(dt16_name)
    nkd16 = NKD - nkd8
    k8 = nkd8 * KPD * P
    x2d = np.ascontiguousarray(x.reshape(M, K)).astype(np.float32, copy=False)
    xt8 = _pretile_x(x2d[:, :k8].astype(NP8), nkd8)
    if nkd16:
        xt16 = _pretile_x(np.ascontiguousarray(x2d[:, k8:]).astype(np16),
                          nkd16)

    # v = 128 * rowsum(x): restores the q-centering exactly (q' = q - 128
    # on device; both the fp8 and fp16 k-ranges are centered).
    v = (128.0 * x2d.astype(np.float64).sum(axis=1)).astype(np.float32)
    vrow = np.ascontiguousarray(v.reshape(NMC * MT, P).T)

    qc = q_int.astype(np.float32) - 128.0   # [-128, 127], exact in f32
    scale_f = scale.astype(np.float32, copy=False)
    bias_f = bias.astype(np.float32, copy=False)

    in_maps = []
    for c in range(NCORES):
        qs = qc[:, c * NS:(c + 1) * NS]
        m = {
            "xt8": xt8,
            "qs8": _pretile_q(np.ascontiguousarray(qs[:k8]).astype(NP8),
                              nkd8),
            "vrow": vrow,
            "scale": np.ascontiguousarray(scale_f[c * NS:(c + 1) * NS]),
            "bias": np.ascontiguousarray(bias_f[c * NS:(c + 1) * NS]),
        }
        if nkd16:
            m["xt16"] = xt16
            m["qs16"] = _pretile_q(
                np.ascontiguousarray(qs[k8:]).astype(np16), nkd16
            )
        in_maps.append(m)
    return in_maps


def _run(x, q_int, scale, zero_point, bias, nkd8=None, dt16_name=None,
         trace=False, **trace_kw):
    nkd8 = NKD8 if nkd8 is None else nkd8
    dt16_name = dt16_name or DT16
    nc = _get_nc(nkd8, dt16_name)
    in_maps = _prep_in_maps(x, q_int, scale, bias, nkd8, dt16_name)
    res = run_bass_kernel_spmd(
        nc, in_maps, list(range(NCORES)), trace=trace, **trace_kw
    )
    out2d = np.concatenate([r["out"] for r in res.results], axis=1)

    if np.any(np.asarray(zero_point) != 0):
        # exact rank-1 correction: -= rowsum(x) ⊗ (scale * zp)
        x2d = x.reshape(M, K).astype(np.float32, copy=False)
        out2d = out2d - np.outer(
            x2d.sum(axis=1),
            scale.astype(np.float32) * zero_point.astype(np.float32),
        )

    return out2d.reshape(B, S, D).astype(np.float32, copy=False), res


def _run_subprocess(x, q_int, scale, zero_point, bias):
    """Fresh-process retry: a NRT_EXEC_UNIT_UNRECOVERABLE poisons the
    in-process PJRT client, but a new process recovers."""
    import os
    import subprocess
    import tempfile

    d = tempfile.mkdtemp(prefix="ffq_retry_")
    names = ["x", "q_int", "scale", "zero_point", "bias"]
    for name, arr in zip(names, [x, q_int, scale, zero_point, bias]):
        np.save(os.path.join(d, name + ".npy"), np.asarray(arr))
    kdir = os.path.dirname(os.path.abspath(__file__))
    code = (
        "import sys, numpy as np\n"
        f"sys.path.insert(0, {kdir!r})\n"
        "import kernel as km\n"
        f"d = {d!r}\n"
        "ins = [np.load(d + '/' + n + '.npy') for n in "
        "['x', 'q_int', 'scale', 'zero_point', 'bias']]\n"
        "out, _ = km._run(*ins)\n"
        "np.save(d + '/out.npy', out)\n"
    )
    subprocess.run([sys.executable, "-c", code], check=True, timeout=2400)
    return np.load(os.path.join(d, "out.npy"))


def kernel(x, q_int, scale, zero_point, bias):
    try:
        out, _ = _run(x, q_int, scale, zero_point, bias)
    except Exception:
        # transient device errors (e.g. a core wedged by a previous
        # profiling session): retry in-process, then in a fresh process
        time.sleep(5)
        try:
            out, _ = _run(x, q_int, scale, zero_point, bias)
        except Exception:
            out = _run_subprocess(x, q_int, scale, zero_point, bias)
    return out


# revision 18
# speedup vs baseline: 1.0433x; 1.0018x over previous
"""FFQLinear Trainium2 kernel (8 NeuronCores, column-parallel, fp8 hybrid).

Computes out = x2d @ W + bias with W = (q_int - zero_point) * scale, where
scale / zero_point broadcast over the OUTPUT-column axis of the [D, D] code
matrix (so W[:, j] = (q[:, j] - zp[j]) * scale[j]).

Math used on device (zp is zero in this problem; a host-side exact rank-1
correction handles the general case): since scale is per-output-column,
    out[:, j] = (x2d @ q)[:, j] * scale[j] + bias[j].

Precision/speed design: the PE runs fp8(e4m3) matmuls at ~2x the 16-bit
rate via perf_mode=DoubleRow (two k-subtiles contracted per instruction).
One pure-fp8 pass has rel err ~2.2e-2 (x-quant 1.6e-2 + q-quant 1.45e-2 in
quadrature) -- just over the 2e-2 budget -- so a split-K hybrid is used:
  - NKD8/8 of K (k-groups of 512) in e4m3 DoubleRow: q is centered
    (q' = q - 128, |q'| <= 128 fits e4m3 with ulp <= 8) and x is cast to
    e4m3. The removed mean is restored exactly in the epilogue via
    out[m, :] += 128 * rowsum(x)[m] (host-computed f32 row sums, added as
    a per-partition scalar before the scale multiply).
  - the remaining (8-NKD8)/8 of K in fp16 (q' ints exact in fp16, x fp16
    rounding ~2e-4): essentially error-free.
With NKD8=6 the simulated rel err is 1.87e-2 (deterministic for the fixed
harness inputs) and PE work drops to 0.25 + 0.75/2 = 62.5% of the fp16
baseline's.

Sharding: column-parallel per the hint. Each of the 8 cores gets
  - x pre-transposed and pre-tiled on the host (contraction dim on SBUF
    partitions, contiguous per-partition DMA lines), replicated
  - a [K, 512] column shard of q', and [512] shards of scale/bias
  - the [M] f32 vector v = 128*rowsum(x), pretiled to per-partition form
and produces a [M, 512] f32 output shard. Host concatenates the shards.
"""

import sys
import time
import types

import numpy as np
import ml_dtypes

import concourse.bass as bass
import concourse.bacc as bacc
import concourse.mybir as mybir
import concourse.tile as tile

# bass_utils' axon trace path does an unguarded
# `from antenv.axon_hooks import get_axon_ntff_profile_hook`; some images
# lack that module. Provide a stub (hook=None -> tracing degrades
# gracefully) so a BASS_TRACE=1 environment can't crash the kernel.
try:
    import antenv.axon_hooks  # noqa: F401
except Exception:
    try:
        import antenv

        _stub = types.ModuleType("antenv.axon_hooks")
        _stub._HOOK = None
        _stub.set_axon_ntff_profile_hook = lambda h: setattr(_stub, "_HOOK", h)
        _stub.get_axon_ntff_profile_hook = lambda: _stub._HOOK
        sys.modules["antenv.axon_hooks"] = _stub
        antenv.axon_hooks = _stub
    except Exception:
        pass

# boot() skips hook registration when the image's antenv lacks axon_hooks;
# with the stub in place, install the same ctypes hook it would have used
# so trace=True yields NTFF profiles / HW exec times.
try:
    import antenv.axon_hooks as _ah

    if _ah.get_axon_ntff_profile_hook() is None:
        from trn_agent_boot.trn_boot import _ntff_profile_via_ctypes

        _hook = _ntff_profile_via_ctypes("/opt/axon/libaxon_pjrt.so")
        if _hook is not None:
            _ah.set_axon_ntff_profile_hook(_hook)
except Exception:
    pass

from concourse.bass_utils import run_bass_kernel_spmd

B, S, D = 2, 2048, 4096
M = B * S            # 4096 output rows
K = D                # 4096 contraction
N = D                # 4096 output cols
NCORES = 8
NS = N // NCORES     # 512 output cols per core

P = 128
KO = K // P          # 32 k-subtiles
M_CHUNK = 512        # rows per chunk (4 psum tiles of 128)
MT = M_CHUNK // P    # 4
NMC = M // M_CHUNK   # 8 m-chunks
KPD = 4              # k-subtiles per x DMA group
NKD = KO // KPD      # 8 k-dma groups total

NKD8 = 6             # k-dma groups done in fp8 DoubleRow (rest fp16)
DT16 = "fp16"        # PE dtype for the high-precision k-groups

F32 = mybir.dt.float32
F8 = mybir.dt.float8e4
NP8 = ml_dtypes.float8_e4m3  # TRN FP8_EXP4-compatible (max 240, RNE)

_CACHE: dict = {}


def _dt16(name: str):
    return mybir.dt.float16 if name == "fp16" else mybir.dt.bfloat16


def _np16(name: str):
    return np.float16 if name == "fp16" else ml_dtypes.bfloat16


def _build(nkd8: int, dt16_name: str) -> bass.Bass:
    assert 1 <= nkd8 <= NKD
    nkd16 = NKD - nkd8
    DT = _dt16(dt16_name)
    DR = mybir.MatmulPerfMode.DoubleRow
    # Bacc (not plain Bass): its compile() runs generate_event_semaphores,
    # which splits multi-wait DMAs to satisfy the 1-wait HW encoding limit.
    nc = bacc.Bacc(
        "TRN2", target_bir_lowering=False, debug=False, num_devices=NCORES
    )
    # Host-pretiled layouts: every DMA below reads a fully-contiguous
    # [P, KPD, *] block.
    xt8 = nc.dram_tensor(
        "xt8", [NMC * nkd8, P, KPD, M_CHUNK], F8, kind="ExternalInput"
    )
    qs8 = nc.dram_tensor("qs8", [nkd8, P, KPD, NS], F8, kind="ExternalInput")
    if nkd16:
        xt16 = nc.dram_tensor(
            "xt16", [NMC * nkd16, P, KPD, M_CHUNK], DT, kind="ExternalInput"
        )
        qs16 = nc.dram_tensor(
            "qs16", [nkd16, P, KPD, NS], DT, kind="ExternalInput"
        )
    vrow_d = nc.dram_tensor("vrow", [P, NMC * MT], F32, kind="ExternalInput")
    scale_d = nc.dram_tensor("scale", [NS], F32, kind="ExternalInput")
    bias_d = nc.dram_tensor("bias", [NS], F32, kind="ExternalInput")
    out_d = nc.dram_tensor("out", [M, NS], F32, kind="ExternalOutput")

    with tile.TileContext(nc) as tc:
        with (
            tc.tile_pool(name="const", bufs=1) as cpool,
            tc.tile_pool(name="x8load", bufs=8) as x8pool,
            tc.tile_pool(name="x0load", bufs=2) as x0pool,
            tc.tile_pool(name="x16load", bufs=4) as x16pool,
            tc.tile_pool(name="opool", bufs=4) as opool,
            tc.tile_pool(name="psum", bufs=8, space="PSUM") as ppool,
        ):
            # Resident q shard. The kd=0 fp8 group is split per k-PAIR
            # (the DoubleRow unit) so the very first matmul waits on a
            # 128KB DMA, not the full group; remaining groups are one
            # DMA each, emitted interleaved with the first m-chunk's x
            # loads.
            q0 = [cpool.tile([P, 2, NS], F8, name=f"q0_{j}") for j in range(2)]
            q8 = [None] + [
                cpool.tile([P, KPD, NS], F8, name=f"q8_{kd}")
                for kd in range(1, nkd8)
            ]
            q16 = [
                cpool.tile([P, KPD, NS], DT, name=f"q16_{kd}")
                for kd in range(nkd16)
            ]
            scale_sb = cpool.tile([P, NS], F32)
            bias_sb = cpool.tile([P, NS], F32)
            v_sb = cpool.tile([P, NMC * MT], F32)

            def rhs8(kd, j):
                return q0[j][:] if kd == 0 else q8[kd][:, 2 * j:2 * j + 2, :]

            def lhs8_of(x8tiles, kd, j, mt):
                t = x8tiles[kd]
                if isinstance(t, list):  # (mc=0, kd=0) pair tiles
                    return t[j][:, :, mt * P:(mt + 1) * P]
                return t[:, 2 * j:2 * j + 2, mt * P:(mt + 1) * P]

            def mm8(psum, x8tiles, kd, j, mt):
                kp = kd * 2 + j
                nc.tensor.matmul(
                    psum[:],
                    lhsT=lhs8_of(x8tiles, kd, j, mt),
                    rhs=rhs8(kd, j),
                    start=(kp == 0),
                    stop=(nkd16 == 0 and kd == nkd8 - 1 and j == 1),
                    perf_mode=DR,
                )

            def mm16(psum, x16tiles, kd, kk, mt):
                nc.tensor.matmul(
                    psum[:],
                    lhsT=x16tiles[kd][:, kk, mt * P:(mt + 1) * P],
                    rhs=q16[kd][:, kk, :],
                    start=False,
                    stop=(kd == nkd16 - 1 and kk == KPD - 1),
                )

            def epilogue(psum, mc, mt, halves=False):
                idx = mc * MT + mt
                row = idx * P
                nh = 2 if halves else 1
                H = NS // nh
                for h in range(nh):
                    # pipeline the very last epilogue in column halves:
                    # DVE on half 1 overlaps the DMA of half 0,
                    # shortening the kernel tail
                    o_sb = opool.tile(
                        [P, H], F32, name=f"osb{nh}{h}", tag=f"o{nh}{h}"
                    )
                    cs = slice(h * H, (h + 1) * H)
                    nc.vector.tensor_scalar_add(
                        o_sb[:], psum[:, cs], v_sb[:, idx:idx + 1]
                    )
                    nc.vector.tensor_mul(o_sb[:], o_sb[:], scale_sb[:, cs])
                    nc.vector.tensor_add(o_sb[:], o_sb[:], bias_sb[:, cs])
                    nc.sync.dma_start(out_d[row:row + P, cs], o_sb[:])

            for mc in range(NMC):
                psums = [
                    ppool.tile([P, NS], F32, name=f"ps{mt}", tag="ps")
                    for mt in range(MT)
                ]
                last_mc = mc == NMC - 1
                first = mc == 0
                x8tiles = []
                x16tiles = []
                for kd in range(nkd8):
                    if first and kd == 0:
                        # per-k-pair 128KB DMAs for the fastest start
                        pairs = []
                        for j in range(2):
                            nc.sync.dma_start(
                                q0[j][:], qs8[0][:, 2 * j:2 * j + 2, :]
                            )
                            x_sb = x0pool.tile(
                                [P, 2, M_CHUNK], F8, name=f"x0_{j}", tag="x0"
                            )
                            nc.sync.dma_start(
                                x_sb[:], xt8[0][:, 2 * j:2 * j + 2, :]
                            )
                            pairs.append(x_sb)
                        x8tiles.append(pairs)
                    else:
                        if first:
                            nc.sync.dma_start(q8[kd][:], qs8[kd])
                        xts = x8pool.tile(
                            [P, KPD, M_CHUNK], F8, name="x8sb", tag="x8"
                        )
                        nc.sync.dma_start(xts[:], xt8[mc * nkd8 + kd])
                        x8tiles.append(xts)
                    if last_mc:
                        continue
                    for j in range(2):
                        for mt in range(MT):
                            mm8(psums[mt], x8tiles, kd, j, mt)
                for kd in range(nkd16):
                    if first:
                        nc.sync.dma_start(q16[kd][:], qs16[kd])
                    xts = x16pool.tile(
                        [P, KPD, M_CHUNK], DT, name="x16sb", tag="x16"
                    )
                    nc.sync.dma_start(xts[:], xt16[mc * nkd16 + kd])
                    x16tiles.append(xts)
                    if last_mc:
                        continue
                    for kk in range(KPD):
                        for mt in range(MT):
                            mm16(psums[mt], x16tiles, kd, kk, mt)
                if last_mc:
                    # mt-major: each psum finishes (and drains through the
                    # epilogue) while later mt groups still compute, so only
                    # one tile's epilogue trails the final matmul.
                    for mt in range(MT):
                        for kd in range(nkd8):
                            for j in range(2):
                                mm8(psums[mt], x8tiles, kd, j, mt)
                        for kd in range(nkd16):
                            for kk in range(KPD):
                                mm16(psums[mt], x16tiles, kd, kk, mt)
                        epilogue(psums[mt], mc, mt, halves=(mt == MT - 1))
                    continue
                if first:
                    nc.sync.dma_start(
                        scale_sb[:], scale_d[None, :].to_broadcast((P, NS))
                    )
                    nc.sync.dma_start(
                        bias_sb[:], bias_d[None, :].to_broadcast((P, NS))
                    )
                    nc.sync.dma_start(v_sb[:], vrow_d[:])
                for mt in range(MT):
                    epilogue(psums[mt], mc, mt)
    nc.compile()
    return nc


def _get_nc(nkd8: int, dt16_name: str) -> bass.Bass:
    key = (nkd8, dt16_name)
    if key not in _CACHE:
        _CACHE[key] = _build(nkd8, dt16_name)
    return _CACHE[key]


def _pretile_x(xpart: np.ndarray, nkd: int) -> np.ndarray:
    """[M, nkd*KPD*P] -> [NMC*nkd, P, KPD, M_CHUNK] with
    XD[mc*nkd+kd, p, kk, m] = xpart[mc*M_CHUNK + m, (kd*KPD+kk)*P + p]."""
    v = xpart.reshape(NMC, M_CHUNK, nkd, KPD, P)
    v = v.transpose(0, 2, 4, 3, 1)  # (mc, kd, p, kk, m)
    return np.ascontiguousarray(v).reshape(NMC * nkd, P, KPD, M_CHUNK)


def _pretile_q(qpart: np.ndarray, nkd: int) -> np.ndarray:
    """[nkd*KPD*P, NS] -> [nkd, P, KPD, NS] with
    QD[kd, p, kk, n] = qpart[(kd*KPD+kk)*P + p, n]."""
    v = qpart.reshape(nkd, KPD, P, NS)
    return np.ascontiguousarray(v.transpose(0, 2, 1, 3))


def _prep_in_maps(x, q_int, scale, bias, nkd8, dt16_name):
    np16 = _np16(dt16_name)
    nkd16 = NKD - nkd8
    k8 = nkd8 * KPD * P
    x2d = np.ascontiguousarray(x.reshape(M, K)).astype(np.float32, copy=False)
    xt8 = _pretile_x(x2d[:, :k8].astype(NP8), nkd8)
    if nkd16:
        xt16 = _pretile_x(np.ascontiguousarray(x2d[:, k8:]).astype(np16),
                          nkd16)

    # v = 128 * rowsum(x): restores the q-centering exactly (q' = q - 128
    # on device; both the fp8 and fp16 k-ranges are centered).
    v = (128.0 * x2d.astype(np.float64).sum(axis=1)).astype(np.float32)
    vrow = np.ascontiguousarray(v.reshape(NMC * MT, P).T)

    qc = q_int.astype(np.float32) - 128.0   # [-128, 127], exact in f32
    scale_f = scale.astype(np.float32, copy=False)
    bias_f = bias.astype(np.float32, copy=False)

    in_maps = []
    for c in range(NCORES):
        qs = qc[:, c * NS:(c + 1) * NS]
        m = {
            "xt8": xt8,
            "qs8": _pretile_q(np.ascontiguousarray(qs[:k8]).astype(NP8),
                              nkd8),
            "vrow": vrow,
            "scale": np.ascontiguousarray(scale_f[c * NS:(c + 1) * NS]),
            "bias": np.ascontiguousarray(bias_f[c * NS:(c + 1) * NS]),
        }
        if nkd16:
            m["xt16"] = xt16
            m["qs16"] = _pretile_q(
                np.ascontiguousarray(qs[k8:]).astype(np16), nkd16
            )
        in_maps.append(m)
    return in_maps


def _run(x, q_int, scale, zero_point, bias, nkd8=None, dt16_name=None,
         trace=False, **trace_kw):
    nkd8 = NKD8 if nkd8 is None else nkd8
    dt16_name = dt16_name or DT16
    nc = _get_nc(nkd8, dt16_name)
    in_maps = _prep_in_maps(x, q_int, scale, bias, nkd8, dt16_name)
    res = run_bass_kernel_spmd(
        nc, in_maps, list(range(NCORES)), trace=trace, **trace_kw
    )
    out2d = np.concatenate([r["out"] for r in res.results], axis=1)

    if np.any(np.asarray(zero_point) != 0):
        # exact rank-1 correction: -= rowsum(x) ⊗ (scale * zp)
        x2d = x.reshape(M, K).astype(np.float32, copy=False)
        out2d = out2d - np.outer(
            x2d.sum(axis=1),
            scale.astype(np.float32) * zero_point.astype(np.float32),
        )

    return out2d.reshape(B, S, D).astype(np.float32, copy=False), res


def _run_subprocess(x, q_int, scale, zero_point, bias):
    """Fresh-process retry: a NRT_EXEC_UNIT_UNRECOVERABLE poisons the
    in-process PJRT client, but a new process recovers."""
    import os
    import subprocess
    import tempfile

    d = tempfile.mkdtemp(prefix="ffq_retry_")
    names = ["x", "q_int", "scale", "zero_point", "bias"]
    for name, arr in zip(names, [x, q_int, scale, zero_point, bias]):
        np.save(os.path.join(d, name + ".npy"), np.asarray(arr))
    kdir = os.path.dirname(os.path.abspath(__file__))
    code = (
        "import sys, numpy as np\n"
        f"sys.path.insert(0, {kdir!r})\n"
        "import kernel as km\n"
        f"d = {d!r}\n"
        "ins = [np.load(d + '/' + n + '.npy') for n in "
        "['x', 'q_int', 'scale', 'zero_point', 'bias']]\n"
        "out, _ = km._run(*ins)\n"
        "np.save(d + '/out.npy', out)\n"
    )
    subprocess.run([sys.executable, "-c", code], check=True, timeout=2400)
    return np.load(os.path.join(d, "out.npy"))


def kernel(x, q_int, scale, zero_point, bias):
    try:
        out, _ = _run(x, q_int, scale, zero_point, bias)
    except Exception:
        # transient device errors (e.g. a core wedged by a previous
        # profiling session): retry in-process, then in a fresh process
        time.sleep(5)
        try:
            out, _ = _run(x, q_int, scale, zero_point, bias)
        except Exception:
            out = _run_subprocess(x, q_int, scale, zero_point, bias)
    return out
